# revision 9
# baseline (speedup 1.0000x reference)
"""GroupWiseTemporalAttention Trainium2 kernel.

Math: in the reference, SDPA runs with seq-len L=S=1 per channel-group, so
softmax over the single key is identically 1 and the attention output equals
v = (x+pe)_group @ v_w.T + v_b.  The whole module therefore folds into one
affine map:

    out = x_flat @ W_eff + b_eff
    W_eff = kron(I_192, v_w.T) @ proj_w.T            # [768, 768]
    b_eff = pe@W_eff + tile(v_b,192)@proj_w.T + proj_b

which we run as a data-parallel GEMM over 8 NeuronCores (6272 rows each).
The per-core kernel streams pre-transposed x^T tiles as the stationary
matmul operand so output lands in natural [tokens, channels] layout.
At bf16 the 128x128 PE array is fully utilized (1 moving column/cycle,
M=K=128), so the stream floor is 49*6*768 columns; everything else below
is about the head (engine boot -> first data), HAM clock ramp, and tail.

Timeline facts (from NTFF traces) this structure is built around:
  * The framework preamble (engine rendezvous + per-engine boot) ends
    ~7us; nothing (DMA issue or PE work) can start earlier.
  * HWDGE issue->first-data is ~1.5us per ring; the scalar (qAct) ring
    historically started ~0.4-1.3us after sync (partly an ACT_TABLE_LOAD
    that rode qAct ahead of the first input -- so NO scalar-engine
    ACTIVATE is used anywhere in this kernel).
  * The PE is HAM-throttled to 1.2 GHz until it has been busy ~3.4us;
    fine-grained (128-col, ~107ns) warm matmuls on a zeroed tile start
    right after the preamble and bridge continuously into the real
    stream so full clock (2.4 GHz) arrives as early as possible.
  * tile0's input and all six W chunks ride ONE packed dram blob ("hd")
    sliced at dependency boundaries, so the lead transfers are few and
    large: [t0a | w0a | t0b | w0b | w1..w5], split across both HWDGE
    rings in strict first-use order.  The first matmul's true deps
    (t0 kc0-2 + w0 cols 0:512) are exactly the first piece per ring.
  * Outputs ride the sync ring; inputs (head tiles then 4-tile blocks)
    ride the scalar ring.  Output is bf16 (halves the write stream);
    host upcasts.
  * The final tile computes 0:512 first (drains via vector + scalar-ring
    DMA while the PE finishes), then 512:768 as two 128-col pieces whose
    add+DMA chains split across both rings, so the post-last-matmul
    critical path is one short DVE add + one small DMA + HBM receipt.
"""

import os

import numpy as np
import ml_dtypes

import concourse.bass as bass
import concourse.mybir as mybir
import concourse.tile as tile
from concourse import bacc
from concourse.bass_utils import run_bass_kernel_spmd

P = 128
C = 768
KC = C // P            # 6 contraction chunks
N_CORES = 8
B, H, W = 16, 56, 56
ROWS = B * H * W       # 50176
RPC = ROWS // N_CORES  # 6272 rows per core
TT = RPC // P          # 49 token tiles per core
TBLK = 4               # token tiles per streamed input DMA block
N_HEAD = 8             # tiles 0..7 DMA'd individually for early availability
NBLK = (TT - N_HEAD - 1) // TBLK  # 10 stream blocks; final tile is its own
N_WARM = 28            # 128-col PE pre-warm matmuls (~115ns each, cold);
                       # sized to bridge the preamble-end (~7.5us) to
                       # first-data (~10.7us) with NO hole: a PE-idle gap
                       # there resets the HAM busy window and delays full
                       # clock by up to 3.4us.  28*115ns = 3.2us < the
                       # 3.41us HAM window, so warm can never overrun the
                       # point where full clock would have arrived anyway.
HD = C + KC * C        # head blob cols: t0 | w0a | t0b | w0b | w1..w5

VARIANT = os.environ.get("GWTA_VARIANT", "bf16")

LAST_STATS: dict = {}

_IN_DT = {
    "bf16": mybir.dt.bfloat16,
    "fp32r": mybir.dt.float32r,
    "fp32": mybir.dt.float32,
}


def _build_nc(variant: str) -> bass.Bass:
    in_dt = _IN_DT[variant]
    nc = bacc.Bacc(None, target_bir_lowering=False)
    # hd: packed head blob [t0a(384) | w0a(512) | t0b(384) | w0b(256) |
    #                      w1(768) | ... | w5(768)]  => [P, 5376]
    hd = nc.declare_dram_parameter("hd", [P, HD], in_dt, isOutput=False)
    # xh: head tiles 1..7 plus the final tile, each [P, KC*P] contiguous.
    xh = nc.declare_dram_parameter("xh", [N_HEAD, P, KC * P], in_dt, isOutput=False)
    xb = nc.declare_dram_parameter(
        "xb", [NBLK, P, KC * TBLK * P], in_dt, isOutput=False
    )
    b = nc.declare_dram_parameter("b", [P, C], mybir.dt.bfloat16, isOutput=False)
    out = nc.declare_dram_parameter(
        "out", [RPC, C], mybir.dt.bfloat16, isOutput=True
    )

    with tile.TileContext(nc) as tc:
        with (
            tc.tile_pool(name="const", bufs=1) as const,
            tc.tile_pool(name="xp", bufs=3) as xp,
            tc.tile_pool(name="op", bufs=6) as op,
            tc.tile_pool(name="pp", bufs=1, space="PSUM") as pp,
        ):
            # PE pre-warm: fine-grained 128-col matmuls on a small zeroed
            # SBUF tile ramp HAM toward full clock during the DMA head so
            # the real stream starts as-unthrottled-as-possible.  They
            # borrow psum slot "pt3", which the real stream touches last.
            g_rhs = const.tile([P, P], in_dt)
            nc.vector.memset(g_rhs[:], 0.0)
            warm = pp.tile([P, C], mybir.dt.float32, tag="pt3")
            for _ in range(N_WARM):
                nc.tensor.matmul(
                    warm[:, 0:P], g_rhs[:], g_rhs[:], start=True, stop=True
                )

            hdt = const.tile([P, HD], in_dt, tag="hd", name="hd")
            xht = [
                const.tile([P, KC * P], in_dt, tag=f"xh{i}", name=f"xh{i}")
                for i in range(N_HEAD - 1)
            ]
            bt = const.tile([P, C], mybir.dt.bfloat16)

            # ---- head DMAs: strict first-use order, split across rings.
            # sync starts data ~0.4us before scalar, so the first piece
            # (the true gate of matmul #0) rides sync.
            nc.sync.dma_start(out=hdt[:, 0:896], in_=hd[:, 0:896])
            nc.scalar.dma_start(out=hdt[:, 896:1536], in_=hd[:, 896:1536])
            nc.sync.dma_start(out=hdt[:, 1536:2304], in_=hd[:, 1536:2304])
            nc.scalar.dma_start(out=hdt[:, 2304:3072], in_=hd[:, 2304:3072])
            nc.sync.dma_start(out=hdt[:, 3072:3840], in_=hd[:, 3072:3840])
            nc.scalar.dma_start(out=hdt[:, 3840:4608], in_=hd[:, 3840:4608])
            nc.scalar.dma_start(out=hdt[:, 4608:5376], in_=hd[:, 4608:5376])
            nc.scalar.dma_start(out=xht[0][:], in_=xh[0])
            nc.scalar.dma_start(out=xht[1][:], in_=xh[1])
            # Bias is only needed once vector adds start; PE is
            # unaffected by a late bias (psum depth 4 absorbs it).
            nc.scalar.dma_start(out=bt[:], in_=b[:])
            for i in range(2, N_HEAD - 1):
                nc.scalar.dma_start(out=xht[i][:], in_=xh[i])
            # Final tile's input, needed last; keep it off the block pool.
            xlt = const.tile([P, KC * P], in_dt, tag="xhl", name="xhl")

            def wA(kc):  # W chunk kc, output cols 0:512
                if kc == 0:
                    return hdt[:, 384:896]
                base = 1536 + (kc - 1) * C
                return hdt[:, base : base + 512]

            def wB(kc):  # W chunk kc, output cols 512:768
                if kc == 0:
                    return hdt[:, 1280:1536]
                base = 1536 + (kc - 1) * C + 512
                return hdt[:, base : base + 256]

            # ---- token-tile loop ----
            for g in range(TT):
                if g == 0:

                    def xsl(kc):
                        if kc < 3:
                            return hdt[:, kc * P : (kc + 1) * P]
                        return hdt[:, 896 + (kc - 3) * P : 896 + (kc - 2) * P]
                elif g < N_HEAD:
                    xt = xht[g - 1]

                    def xsl(kc, xt=xt):
                        return xt[:, kc * P : (kc + 1) * P]
                elif g == TT - 1:
                    nc.scalar.dma_start(out=xlt[:], in_=xh[N_HEAD - 1])

                    def xsl(kc):
                        return xlt[:, kc * P : (kc + 1) * P]
                else:
                    bi, s = divmod(g - N_HEAD, TBLK)
                    if s == 0:
                        xbt = xp.tile(
                            [P, KC, TBLK * P], in_dt, tag="xb", name="xb"
                        )
                        nc.scalar.dma_start(
                            out=xbt[:],
                            in_=xb[bi].rearrange(
                                "p (kc t) -> p kc t", kc=KC
                            ),
                        )

                    def xsl(kc, xbt=xbt, s=s):
                        return xbt[:, kc, s * P : (s + 1) * P]

                pt = pp.tile(
                    [P, C], mybir.dt.float32, tag=f"pt{g % 4}", name=f"pt{g % 4}"
                )
                ot = op.tile([P, C], mybir.dt.bfloat16, tag="ot")
                row = slice(g * P, (g + 1) * P)
                if g == TT - 1:
                    # Final tile: 0:512 half computed FIRST so its vector
                    # add + scalar-ring DMA drain while the 512:768 half
                    # is still on the PE; the 512:768 half runs as two
                    # 128-col pieces whose add+DMA chains split across
                    # both rings, minimizing the post-last-matmul path.
                    # The halves use DIFFERENT psum tags so the second
                    # half's matmuls carry no WAR dependency on the adds.
                    pt2 = pp.tile(
                        [P, C], mybir.dt.float32,
                        tag=f"pt{(g + 1) % 4}", name=f"pt{(g + 1) % 4}",
                    )
                    for kc in range(KC):
                        nc.tensor.matmul(
                            pt2[:, 0:512], xsl(kc), wA(kc),
                            start=(kc == 0), stop=(kc == KC - 1),
                        )
                    nc.vector.tensor_add(
                        out=ot[:, 0:512], in0=pt2[:, 0:512], in1=bt[:, 0:512]
                    )
                    nc.scalar.dma_start(out=out[row, 0:512], in_=ot[:, 0:512])
                    # 512:768 must be ONE accumulation group (both 128-col
                    # sub-ranges share a PSUM bank == one zero region; two
                    # groups there are illegal).  Split only the post-stop
                    # add+DMA chains across both rings.
                    for kc in range(KC):
                        nc.tensor.matmul(
                            pt[:, 512:C], xsl(kc), wB(kc),
                            start=(kc == 0), stop=(kc == KC - 1),
                        )
                    nc.vector.tensor_add(
                        out=ot[:, 512:640], in0=pt[:, 512:640], in1=bt[:, 512:640]
                    )
                    nc.sync.dma_start(out=out[row, 512:640], in_=ot[:, 512:640])
                    nc.vector.tensor_add(
                        out=ot[:, 640:768], in0=pt[:, 640:768], in1=bt[:, 640:768]
                    )
                    nc.scalar.dma_start(out=out[row, 640:768], in_=ot[:, 640:768])
                    continue
                for kc in range(KC):
                    lhsT = xsl(kc)
                    nc.tensor.matmul(
                        pt[:, 0:512], lhsT, wA(kc),
                        start=(kc == 0), stop=(kc == KC - 1),
                    )
                    nc.tensor.matmul(
                        pt[:, 512:C], lhsT, wB(kc),
                        start=(kc == 0), stop=(kc == KC - 1),
                    )

                if g == TT - 2:
                    # Tail drain: per-half add + DMA, halves split across
                    # BOTH rings so issue (~0.6us/instr) and completion
                    # receipts run in parallel.
                    nc.vector.tensor_add(
                        out=ot[:, 0:512], in0=pt[:, 0:512], in1=bt[:, 0:512]
                    )
                    nc.scalar.dma_start(out=out[row, 0:512], in_=ot[:, 0:512])
                    nc.vector.tensor_add(
                        out=ot[:, 512:C], in0=pt[:, 512:C], in1=bt[:, 512:C]
                    )
                    nc.sync.dma_start(out=out[row, 512:C], in_=ot[:, 512:C])
                else:
                    # split at the PSUM bank boundary (one bank per read)
                    nc.vector.tensor_add(
                        out=ot[:, 0:512], in0=pt[:, 0:512], in1=bt[:, 0:512]
                    )
                    nc.vector.tensor_add(
                        out=ot[:, 512:C], in0=pt[:, 512:C], in1=bt[:, 512:C]
                    )
                    nc.sync.dma_start(out=out[row, :], in_=ot[:])
    nc.compile()
    return nc


def _fold_weights(qkv_w, qkv_b, proj_w, proj_b, pe):
    v_w = qkv_w[2 * 4 : 3 * 4].astype(np.float64)   # [4, 4]
    v_b = qkv_b[2 * 4 : 3 * 4].astype(np.float64)   # [4]
    bd = np.kron(np.eye(C // 4), v_w.T)             # y_flat @ bd == groupwise v
    w_eff = bd @ proj_w.astype(np.float64).T        # [768, 768]
    b_eff = (
        np.tile(v_b, C // 4) @ proj_w.astype(np.float64).T
        + proj_b.astype(np.float64)
        + pe[:C].astype(np.float64) @ w_eff
    )
    return w_eff, b_eff


def _enable_tracing_shims():
    """Dev-only (GWTA_TRACE=1): restore the NTFF profile hook that this
    image's `antenv` is missing, and keep trace artifacts local instead of
    uploading.  Never active when the kernel is called normally."""
    import sys
    import types

    try:
        from antenv import axon_hooks  # noqa: F401
    except ImportError:
        import antenv
        from trn_agent_boot.trn_boot import _ntff_profile_via_ctypes

        mod = types.ModuleType("antenv.axon_hooks")
        mod._hook = _ntff_profile_via_ctypes("/opt/axon/libaxon_pjrt.so")
        mod.get_axon_ntff_profile_hook = lambda: mod._hook
        mod.set_axon_ntff_profile_hook = lambda h: setattr(mod, "_hook", h)
        sys.modules["antenv.axon_hooks"] = mod
        antenv.axon_hooks = mod

    import concourse.bass_utils as bu

    bu.upload_artifacts = lambda tmpdir: f"local:{tmpdir}"


def kernel(x, qkv_w, qkv_b, proj_w, proj_b, pe):
    x = np.asarray(x, np.float32)
    w_eff, b_eff = _fold_weights(
        np.asarray(qkv_w), np.asarray(qkv_b),
        np.asarray(proj_w), np.asarray(proj_b), np.asarray(pe),
    )

    variant = VARIANT
    if variant == "bf16":
        cast = lambda a: np.ascontiguousarray(a, dtype=ml_dtypes.bfloat16)
    else:
        cast = lambda a: np.ascontiguousarray(a, dtype=np.float32)

    # W packed partition-major: (p, kc, j) = W_eff[kc*128+p, j]
    w_dev = np.ascontiguousarray(
        cast(w_eff).reshape(KC, P, C).transpose(1, 0, 2)
    ).reshape(P, KC * C)
    b_dev = np.broadcast_to(
        b_eff.astype(ml_dtypes.bfloat16), (P, C)
    ).copy()

    x_flat = x.reshape(ROWS, C)
    in_maps = []
    head_tiles = list(range(1, N_HEAD)) + [TT - 1]
    for c in range(N_CORES):
        xT = cast(x_flat[c * RPC : (c + 1) * RPC].T)   # [C, RPC]
        xr = xT.reshape(KC, P, RPC)
        t0p = np.ascontiguousarray(
            xr[:, :, 0:P].transpose(1, 0, 2)
        ).reshape(P, KC * P)
        hd_dev = np.ascontiguousarray(
            np.concatenate(
                [
                    t0p[:, 0:384],
                    w_dev[:, 0:512],
                    t0p[:, 384:768],
                    w_dev[:, 512:768],
                    w_dev[:, 768:],
                ],
                axis=1,
            )
        )
        xh_dev = np.ascontiguousarray(
            np.stack(
                [xr[:, :, t * P : (t + 1) * P] for t in head_tiles], axis=0
            ).transpose(0, 2, 1, 3)
        ).reshape(N_HEAD, P, KC * P)
        xb_dev = np.ascontiguousarray(
            xr[:, :, N_HEAD * P : (TT - 1) * P]
            .reshape(KC, P, NBLK, TBLK * P)
            .transpose(2, 1, 0, 3)
        ).reshape(NBLK, P, KC * TBLK * P)
        in_maps.append(
            {"hd": hd_dev, "xh": xh_dev, "xb": xb_dev, "b": b_dev}
        )

    nc = _build_nc(variant)
    trace = bool(int(os.environ.get("GWTA_TRACE", "0")))
    kw = {}
    if trace:
        _enable_tracing_shims()
        kw["tmpdir"] = os.environ.get("GWTA_TRACE_DIR") or None
    r = run_bass_kernel_spmd(nc, in_maps, list(range(N_CORES)), trace=trace, **kw)

    LAST_STATS.clear()
    LAST_STATS.update(
        exec_time_ns=r.exec_time_ns,
        mean_exec_time_ns=r.mean_exec_time_ns,
        variant=variant,
    )

    out = np.empty((ROWS, C), np.float32)
    for c in range(N_CORES):
        out[c * RPC : (c + 1) * RPC] = np.asarray(
            r.results[c]["out"]
        ).astype(np.float32)
    return out.reshape(B, H, W, C)


# revision 11
# speedup vs baseline: 1.1673x; 1.1673x over previous
"""GroupWiseTemporalAttention Trainium2 kernel.

Math: in the reference, SDPA runs with seq-len L=S=1 per channel-group, so
softmax over the single key is identically 1 and the attention output equals
v = (x+pe)_group @ v_w.T + v_b.  The whole module therefore folds into one
affine map:

    out = x_flat @ W_eff + b_eff
    W_eff = kron(I_192, v_w.T) @ proj_w.T            # [768, 768]
    b_eff = pe@W_eff + tile(v_b,192)@proj_w.T + proj_b

which we run as a data-parallel GEMM over 8 NeuronCores (6272 rows each).
The per-core kernel streams pre-transposed x^T tiles as the stationary
matmul operand so output lands in natural [tokens, channels] layout.
At bf16 the 128x128 PE array is fully utilized (1 moving column/cycle,
M=K=128), so the stream floor is 49*6*768 columns; everything else below
is about the head (engine boot -> first data), HAM clock ramp, and tail.

Timeline facts (from NTFF traces) this structure is built around:
  * The framework preamble (engine rendezvous + per-engine boot) ends
    ~7us; nothing (DMA issue or PE work) can start earlier.
  * HWDGE issue->first-data is ~1.5us per ring; the scalar (qAct) ring
    historically started ~0.4-1.3us after sync (partly an ACT_TABLE_LOAD
    that rode qAct ahead of the first input -- so NO scalar-engine
    ACTIVATE is used anywhere in this kernel).
  * The PE is HAM-throttled to 1.2 GHz until it has been busy ~3.4us;
    fine-grained (128-col, ~107ns) warm matmuls on a zeroed tile start
    right after the preamble and bridge continuously into the real
    stream so full clock (2.4 GHz) arrives as early as possible.
  * tile0's input and all six W chunks ride ONE packed dram blob ("hd")
    sliced at dependency boundaries, so the lead transfers are few and
    large: [t0a | w0a | t0b | w0b | w1..w5], split across both HWDGE
    rings in strict first-use order.  The first matmul's true deps
    (t0 kc0-2 + w0 cols 0:512) are exactly the first piece per ring.
  * Outputs ride the sync ring; inputs (head tiles then 4-tile blocks)
    ride the scalar ring.  Output is bf16 (halves the write stream);
    host upcasts.
  * The final tile computes 0:512 first (drains via vector + scalar-ring
    DMA while the PE finishes), then 512:768 as two 128-col pieces whose
    add+DMA chains split across both rings, so the post-last-matmul
    critical path is one short DVE add + one small DMA + HBM receipt.
"""

import os

import numpy as np
import ml_dtypes

import concourse.bass as bass
import concourse.mybir as mybir
import concourse.tile as tile
from concourse import bacc
from concourse.bass_utils import run_bass_kernel_spmd

P = 128
C = 768
KC = C // P            # 6 contraction chunks
N_CORES = 8
B, H, W = 16, 56, 56
ROWS = B * H * W       # 50176
RPC = ROWS // N_CORES  # 6272 rows per core
TT = RPC // P          # 49 token tiles per core
TBLK = 4               # token tiles per streamed input DMA block
N_HEAD = 8             # tiles 0..7 DMA'd individually for early availability
NBLK = (TT - N_HEAD - 1) // TBLK  # 10 stream blocks; final tile is its own
N_WARM = 24            # 128-col PE pre-warm matmuls (~135ns each, cold);
                       # sized to bridge the preamble-end (~7.7us) to
                       # first-data (~10.7-11us) with NO hole: a PE-idle
                       # gap there resets the HAM busy window and delays
                       # full clock by up to 3.4us, while overrun only
                       # costs 1:1 -- so err slightly long.
HD = C + KC * C        # head blob cols: t0 | w0a | t0b | w0b | w1..w5

VARIANT = os.environ.get("GWTA_VARIANT", "bf16")

LAST_STATS: dict = {}

_IN_DT = {
    "bf16": mybir.dt.bfloat16,
    "fp32r": mybir.dt.float32r,
    "fp32": mybir.dt.float32,
}


def _build_nc(variant: str) -> bass.Bass:
    in_dt = _IN_DT[variant]
    nc = bacc.Bacc(None, target_bir_lowering=False)
    # hd: packed head blob [t0a(384) | w0a(512) | t0b(384) | w0b(256) |
    #                      w1(768) | ... | w5(768)]  => [P, 5376]
    hd = nc.declare_dram_parameter("hd", [P, HD], in_dt, isOutput=False)
    # xh: head tiles 1..7 plus the final tile, each [P, KC*P] contiguous.
    xh = nc.declare_dram_parameter("xh", [N_HEAD, P, KC * P], in_dt, isOutput=False)
    xb = nc.declare_dram_parameter(
        "xb", [NBLK, P, KC * TBLK * P], in_dt, isOutput=False
    )
    b = nc.declare_dram_parameter("b", [P, C], mybir.dt.bfloat16, isOutput=False)
    out = nc.declare_dram_parameter(
        "out", [RPC, C], mybir.dt.bfloat16, isOutput=True
    )

    with tile.TileContext(nc) as tc:
        with (
            tc.tile_pool(name="const", bufs=1) as const,
            tc.tile_pool(name="xp", bufs=3) as xp,
            tc.tile_pool(name="op", bufs=6) as op,
            tc.tile_pool(name="pp", bufs=1, space="PSUM") as pp,
        ):
            # PE pre-warm: fine-grained 128-col matmuls on a small zeroed
            # SBUF tile ramp HAM toward full clock during the DMA head so
            # the real stream starts as-unthrottled-as-possible.  They
            # borrow psum slot "pt3", which the real stream touches last.
            g_rhs = const.tile([P, P], in_dt)
            nc.vector.memset(g_rhs[:], 0.0)
            warm = pp.tile([P, C], mybir.dt.float32, tag="pt3")
            for _ in range(N_WARM):
                nc.tensor.matmul(
                    warm[:, 0:P], g_rhs[:], g_rhs[:], start=True, stop=True
                )

            hdt = const.tile([P, HD], in_dt, tag="hd", name="hd")
            xht = [
                const.tile([P, KC * P], in_dt, tag=f"xh{i}", name=f"xh{i}")
                for i in range(N_HEAD - 1)
            ]
            bt = const.tile([P, C], mybir.dt.bfloat16)

            # ---- head DMAs: strict first-use order, ALTERNATING rings so
            # consecutive dependencies travel in parallel.  Matmul #0's two
            # deps are exactly piece A1 (t0a, sync) + piece B1 (w0a,
            # scalar); the early wire rate is only ~65-130GB/s per ring,
            # so putting them on one ring would serialize ~1us.
            nc.sync.dma_start(out=hdt[:, 0:384], in_=hd[:, 0:384])        # t0a
            nc.scalar.dma_start(out=hdt[:, 384:896], in_=hd[:, 384:896])  # w0a
            nc.sync.dma_start(out=hdt[:, 896:1536], in_=hd[:, 896:1536])  # t0b+w0b
            nc.scalar.dma_start(out=hdt[:, 1536:2304], in_=hd[:, 1536:2304])  # w1
            nc.sync.dma_start(out=hdt[:, 2304:3072], in_=hd[:, 2304:3072])    # w2
            nc.scalar.dma_start(out=hdt[:, 3072:3840], in_=hd[:, 3072:3840])  # w3
            nc.sync.dma_start(out=hdt[:, 3840:4608], in_=hd[:, 3840:4608])    # w4
            nc.scalar.dma_start(out=hdt[:, 4608:5376], in_=hd[:, 4608:5376])  # w5
            nc.scalar.dma_start(out=xht[0][:], in_=xh[0])
            nc.scalar.dma_start(out=xht[1][:], in_=xh[1])
            # Bias is only needed once vector adds start; PE is
            # unaffected by a late bias (psum depth 4 absorbs it).
            nc.scalar.dma_start(out=bt[:], in_=b[:])
            for i in range(2, N_HEAD - 1):
                nc.scalar.dma_start(out=xht[i][:], in_=xh[i])
            # Final tile's input, needed last; keep it off the block pool.
            xlt = const.tile([P, KC * P], in_dt, tag="xhl", name="xhl")

            def wA(kc):  # W chunk kc, output cols 0:512
                if kc == 0:
                    return hdt[:, 384:896]
                base = 1536 + (kc - 1) * C
                return hdt[:, base : base + 512]

            def wB(kc):  # W chunk kc, output cols 512:768
                if kc == 0:
                    return hdt[:, 1280:1536]
                base = 1536 + (kc - 1) * C + 512
                return hdt[:, base : base + 256]

            # ---- token-tile loop ----
            for g in range(TT):
                if g == 0:

                    def xsl(kc):
                        if kc < 3:
                            return hdt[:, kc * P : (kc + 1) * P]
                        return hdt[:, 896 + (kc - 3) * P : 896 + (kc - 2) * P]
                elif g < N_HEAD:
                    xt = xht[g - 1]

                    def xsl(kc, xt=xt):
                        return xt[:, kc * P : (kc + 1) * P]
                elif g == TT - 1:
                    nc.scalar.dma_start(out=xlt[:], in_=xh[N_HEAD - 1])

                    def xsl(kc):
                        return xlt[:, kc * P : (kc + 1) * P]
                else:
                    bi, s = divmod(g - N_HEAD, TBLK)
                    if s == 0:
                        xbt = xp.tile(
                            [P, KC, TBLK * P], in_dt, tag="xb", name="xb"
                        )
                        nc.scalar.dma_start(
                            out=xbt[:],
                            in_=xb[bi].rearrange(
                                "p (kc t) -> p kc t", kc=KC
                            ),
                        )

                    def xsl(kc, xbt=xbt, s=s):
                        return xbt[:, kc, s * P : (s + 1) * P]

                pt = pp.tile(
                    [P, C], mybir.dt.float32, tag=f"pt{g % 4}", name=f"pt{g % 4}"
                )
                ot = op.tile([P, C], mybir.dt.bfloat16, tag="ot")
                row = slice(g * P, (g + 1) * P)
                if g == TT - 1:
                    # Final tile: 0:512 half computed FIRST so its vector
                    # add + scalar-ring DMA drain while the 512:768 half
                    # is still on the PE; the 512:768 half runs as two
                    # 128-col pieces whose add+DMA chains split across
                    # both rings, minimizing the post-last-matmul path.
                    # The halves use DIFFERENT psum tags so the second
                    # half's matmuls carry no WAR dependency on the adds.
                    pt2 = pp.tile(
                        [P, C], mybir.dt.float32,
                        tag=f"pt{(g + 1) % 4}", name=f"pt{(g + 1) % 4}",
                    )
                    for kc in range(KC):
                        nc.tensor.matmul(
                            pt2[:, 0:512], xsl(kc), wA(kc),
                            start=(kc == 0), stop=(kc == KC - 1),
                        )
                    nc.vector.tensor_add(
                        out=ot[:, 0:512], in0=pt2[:, 0:512], in1=bt[:, 0:512]
                    )
                    nc.scalar.dma_start(out=out[row, 0:512], in_=ot[:, 0:512])
                    # 512:768 must be ONE accumulation group (both 128-col
                    # sub-ranges share a PSUM bank == one zero region; two
                    # groups there are illegal).  Split only the post-stop
                    # add+DMA chains across both rings.
                    for kc in range(KC):
                        nc.tensor.matmul(
                            pt[:, 512:C], xsl(kc), wB(kc),
                            start=(kc == 0), stop=(kc == KC - 1),
                        )
                    nc.vector.tensor_add(
                        out=ot[:, 512:640], in0=pt[:, 512:640], in1=bt[:, 512:640]
                    )
                    nc.sync.dma_start(out=out[row, 512:640], in_=ot[:, 512:640])
                    nc.vector.tensor_add(
                        out=ot[:, 640:768], in0=pt[:, 640:768], in1=bt[:, 640:768]
                    )
                    nc.scalar.dma_start(out=out[row, 640:768], in_=ot[:, 640:768])
                    continue
                for kc in range(KC):
                    lhsT = xsl(kc)
                    nc.tensor.matmul(
                        pt[:, 0:512], lhsT, wA(kc),
                        start=(kc == 0), stop=(kc == KC - 1),
                    )
                    nc.tensor.matmul(
                        pt[:, 512:C], lhsT, wB(kc),
                        start=(kc == 0), stop=(kc == KC - 1),
                    )

                if g == TT - 2:
                    # Tail drain: per-half add + DMA, halves split across
                    # BOTH rings so issue (~0.6us/instr) and completion
                    # receipts run in parallel.
                    nc.vector.tensor_add(
                        out=ot[:, 0:512], in0=pt[:, 0:512], in1=bt[:, 0:512]
                    )
                    nc.scalar.dma_start(out=out[row, 0:512], in_=ot[:, 0:512])
                    nc.vector.tensor_add(
                        out=ot[:, 512:C], in0=pt[:, 512:C], in1=bt[:, 512:C]
                    )
                    nc.sync.dma_start(out=out[row, 512:C], in_=ot[:, 512:C])
                else:
                    # split at the PSUM bank boundary (one bank per read)
                    nc.vector.tensor_add(
                        out=ot[:, 0:512], in0=pt[:, 0:512], in1=bt[:, 0:512]
                    )
                    nc.vector.tensor_add(
                        out=ot[:, 512:C], in0=pt[:, 512:C], in1=bt[:, 512:C]
                    )
                    nc.sync.dma_start(out=out[row, :], in_=ot[:])
    nc.compile()
    return nc


def _fold_weights(qkv_w, qkv_b, proj_w, proj_b, pe):
    v_w = qkv_w[2 * 4 : 3 * 4].astype(np.float64)   # [4, 4]
    v_b = qkv_b[2 * 4 : 3 * 4].astype(np.float64)   # [4]
    bd = np.kron(np.eye(C // 4), v_w.T)             # y_flat @ bd == groupwise v
    w_eff = bd @ proj_w.astype(np.float64).T        # [768, 768]
    b_eff = (
        np.tile(v_b, C // 4) @ proj_w.astype(np.float64).T
        + proj_b.astype(np.float64)
        + pe[:C].astype(np.float64) @ w_eff
    )
    return w_eff, b_eff


def _enable_tracing_shims():
    """Dev-only (GWTA_TRACE=1): restore the NTFF profile hook that this
    image's `antenv` is missing, and keep trace artifacts local instead of
    uploading.  Never active when the kernel is called normally."""
    import sys
    import types

    try:
        from antenv import axon_hooks  # noqa: F401
    except ImportError:
        import antenv
        from trn_agent_boot.trn_boot import _ntff_profile_via_ctypes

        mod = types.ModuleType("antenv.axon_hooks")
        mod._hook = _ntff_profile_via_ctypes("/opt/axon/libaxon_pjrt.so")
        mod.get_axon_ntff_profile_hook = lambda: mod._hook
        mod.set_axon_ntff_profile_hook = lambda h: setattr(mod, "_hook", h)
        sys.modules["antenv.axon_hooks"] = mod
        antenv.axon_hooks = mod

    import concourse.bass_utils as bu

    bu.upload_artifacts = lambda tmpdir: f"local:{tmpdir}"


def kernel(x, qkv_w, qkv_b, proj_w, proj_b, pe):
    x = np.asarray(x, np.float32)
    w_eff, b_eff = _fold_weights(
        np.asarray(qkv_w), np.asarray(qkv_b),
        np.asarray(proj_w), np.asarray(proj_b), np.asarray(pe),
    )

    variant = VARIANT
    if variant == "bf16":
        cast = lambda a: np.ascontiguousarray(a, dtype=ml_dtypes.bfloat16)
    else:
        cast = lambda a: np.ascontiguousarray(a, dtype=np.float32)

    # W packed partition-major: (p, kc, j) = W_eff[kc*128+p, j]
    w_dev = np.ascontiguousarray(
        cast(w_eff).reshape(KC, P, C).transpose(1, 0, 2)
    ).reshape(P, KC * C)
    b_dev = np.broadcast_to(
        b_eff.astype(ml_dtypes.bfloat16), (P, C)
    ).copy()

    x_flat = x.reshape(ROWS, C)
    in_maps = []
    head_tiles = list(range(1, N_HEAD)) + [TT - 1]
    for c in range(N_CORES):
        xT = cast(x_flat[c * RPC : (c + 1) * RPC].T)   # [C, RPC]
        xr = xT.reshape(KC, P, RPC)
        t0p = np.ascontiguousarray(
            xr[:, :, 0:P].transpose(1, 0, 2)
        ).reshape(P, KC * P)
        hd_dev = np.ascontiguousarray(
            np.concatenate(
                [
                    t0p[:, 0:384],
                    w_dev[:, 0:512],
                    t0p[:, 384:768],
                    w_dev[:, 512:768],
                    w_dev[:, 768:],
                ],
                axis=1,
            )
        )
        xh_dev = np.ascontiguousarray(
            np.stack(
                [xr[:, :, t * P : (t + 1) * P] for t in head_tiles], axis=0
            ).transpose(0, 2, 1, 3)
        ).reshape(N_HEAD, P, KC * P)
        xb_dev = np.ascontiguousarray(
            xr[:, :, N_HEAD * P : (TT - 1) * P]
            .reshape(KC, P, NBLK, TBLK * P)
            .transpose(2, 1, 0, 3)
        ).reshape(NBLK, P, KC * TBLK * P)
        in_maps.append(
            {"hd": hd_dev, "xh": xh_dev, "xb": xb_dev, "b": b_dev}
        )

    nc = _build_nc(variant)
    trace = bool(int(os.environ.get("GWTA_TRACE", "0")))
    kw = {}
    if trace:
        _enable_tracing_shims()
        kw["tmpdir"] = os.environ.get("GWTA_TRACE_DIR") or None
    r = run_bass_kernel_spmd(nc, in_maps, list(range(N_CORES)), trace=trace, **kw)

    LAST_STATS.clear()
    LAST_STATS.update(
        exec_time_ns=r.exec_time_ns,
        mean_exec_time_ns=r.mean_exec_time_ns,
        variant=variant,
    )

    out = np.empty((ROWS, C), np.float32)
    for c in range(N_CORES):
        out[c * RPC : (c + 1) * RPC] = np.asarray(
            r.results[c]["out"]
        ).astype(np.float32)
    return out.reshape(B, H, W, C)


# revision 13
# speedup vs baseline: 1.1719x; 1.0039x over previous
"""GroupWiseTemporalAttention Trainium2 kernel.

Math: in the reference, SDPA runs with seq-len L=S=1 per channel-group, so
softmax over the single key is identically 1 and the attention output equals
v = (x+pe)_group @ v_w.T + v_b.  The whole module therefore folds into one
affine map:

    out = x_flat @ W_eff + b_eff
    W_eff = kron(I_192, v_w.T) @ proj_w.T            # [768, 768]
    b_eff = pe@W_eff + tile(v_b,192)@proj_w.T + proj_b

which we run as a data-parallel GEMM over 8 NeuronCores (6272 rows each).
The per-core kernel streams pre-transposed x^T tiles as the stationary
matmul operand so output lands in natural [tokens, channels] layout.
At bf16 the 128x128 PE array is fully utilized (1 moving column/cycle,
M=K=128), so the stream floor is 49*6*768 columns; everything else below
is about the head (engine boot -> first data), HAM clock ramp, and tail.

Timeline facts (from NTFF traces) this structure is built around:
  * The framework preamble (engine rendezvous + per-engine boot) ends
    ~7us; nothing (DMA issue or PE work) can start earlier.
  * HWDGE issue->first-data is ~1.5us per ring; the scalar (qAct) ring
    historically started ~0.4-1.3us after sync (partly an ACT_TABLE_LOAD
    that rode qAct ahead of the first input -- so NO scalar-engine
    ACTIVATE is used anywhere in this kernel).
  * The PE is HAM-throttled to 1.2 GHz until it has been busy ~3.4us;
    fine-grained (128-col, ~107ns) warm matmuls on a zeroed tile start
    right after the preamble and bridge continuously into the real
    stream so full clock (2.4 GHz) arrives as early as possible.
  * tile0's input and all six W chunks ride ONE packed dram blob ("hd")
    sliced at dependency boundaries, so the lead transfers are few and
    large: [t0a | w0a | t0b | w0b | w1..w5], split across both HWDGE
    rings in strict first-use order.  The first matmul's true deps
    (t0 kc0-2 + w0 cols 0:512) are exactly the first piece per ring.
  * Outputs ride the sync ring; inputs (head tiles then 4-tile blocks)
    ride the scalar ring.  Output is bf16 (halves the write stream);
    host upcasts.
  * The final tile computes 0:512 first (drains via vector + scalar-ring
    DMA while the PE finishes), then 512:768 as two 128-col pieces whose
    add+DMA chains split across both rings, so the post-last-matmul
    critical path is one short DVE add + one small DMA + HBM receipt.
"""

import os

import numpy as np
import ml_dtypes

import concourse.bass as bass
import concourse.mybir as mybir
import concourse.tile as tile
from concourse import bacc
from concourse.bass_utils import run_bass_kernel_spmd

P = 128
C = 768
KC = C // P            # 6 contraction chunks
N_CORES = 8
B, H, W = 16, 56, 56
ROWS = B * H * W       # 50176
RPC = ROWS // N_CORES  # 6272 rows per core
TT = RPC // P          # 49 token tiles per core
TBLK = 4               # token tiles per streamed input DMA block
N_HEAD = 8             # tiles 0..7 DMA'd individually for early availability
NBLK = (TT - N_HEAD - 1) // TBLK  # 10 stream blocks; final tile is its own
N_WARM = 34            # 128-col PE pre-warm matmuls (~114ns cold, ~57ns
                       # once HAM fires at warm_start+3.4us).  Sized so
                       # warm ends ~11.2us, past the typical first-data
                       # time: a PE-idle hole between warm and the real
                       # stream resets/oscillates the HAM busy window
                       # (costs 1.7-3us), while overrun past the HAM fire
                       # point costs only ~57ns per excess matmul.
HD = C + KC * C        # head blob cols: t0 | w0a | t0b | w0b | w1..w5

VARIANT = os.environ.get("GWTA_VARIANT", "bf16")

LAST_STATS: dict = {}

_IN_DT = {
    "bf16": mybir.dt.bfloat16,
    "fp32r": mybir.dt.float32r,
    "fp32": mybir.dt.float32,
}


def _build_nc(variant: str) -> bass.Bass:
    in_dt = _IN_DT[variant]
    nc = bacc.Bacc(None, target_bir_lowering=False)
    # hd: packed head blob [t0a(384) | w0a(512) | t0b(384) | w0b(256) |
    #                      w1(768) | ... | w5(768)]  => [P, 5376]
    hd = nc.declare_dram_parameter("hd", [P, HD], in_dt, isOutput=False)
    # xh: head tiles 1..7 plus the final tile, each [P, KC*P] contiguous.
    xh = nc.declare_dram_parameter("xh", [N_HEAD, P, KC * P], in_dt, isOutput=False)
    xb = nc.declare_dram_parameter(
        "xb", [NBLK, P, KC * TBLK * P], in_dt, isOutput=False
    )
    b = nc.declare_dram_parameter("b", [P, C], mybir.dt.bfloat16, isOutput=False)
    out = nc.declare_dram_parameter(
        "out", [RPC, C], mybir.dt.bfloat16, isOutput=True
    )

    with tile.TileContext(nc) as tc:
        with (
            tc.tile_pool(name="const", bufs=1) as const,
            tc.tile_pool(name="xp", bufs=3) as xp,
            tc.tile_pool(name="op", bufs=6) as op,
            tc.tile_pool(name="pp", bufs=1, space="PSUM") as pp,
        ):
            # PE pre-warm: fine-grained 128-col matmuls on a small zeroed
            # SBUF tile ramp HAM toward full clock during the DMA head so
            # the real stream starts as-unthrottled-as-possible.  They
            # borrow psum slot "pt3", which the real stream touches last.
            g_rhs = const.tile([P, P], in_dt)
            nc.vector.memset(g_rhs[:], 0.0)
            warm = pp.tile([P, C], mybir.dt.float32, tag="pt3")
            for _ in range(N_WARM):
                nc.tensor.matmul(
                    warm[:, 0:P], g_rhs[:], g_rhs[:], start=True, stop=True
                )

            hdt = const.tile([P, HD], in_dt, tag="hd", name="hd")
            xht = [
                const.tile([P, KC * P], in_dt, tag=f"xh{i}", name=f"xh{i}")
                for i in range(N_HEAD - 1)
            ]
            bt = const.tile([P, C], mybir.dt.bfloat16)

            # ---- head DMAs: strict first-use order.  The scalar (qAct)
            # ring's first data consistently lags sync's by 0.4-1.5us, so
            # matmul #0's deps (t0a then w0a) BOTH lead the sync ring --
            # its FIFO delivers them back-to-back and the first real MM
            # never waits on the late ring.  Later chunks alternate.
            nc.sync.dma_start(out=hdt[:, 0:384], in_=hd[:, 0:384])        # t0a
            nc.sync.dma_start(out=hdt[:, 384:896], in_=hd[:, 384:896])    # w0a
            nc.sync.dma_start(out=hdt[:, 896:1536], in_=hd[:, 896:1536])  # t0b+w0b
            nc.scalar.dma_start(out=hdt[:, 1536:2304], in_=hd[:, 1536:2304])  # w1
            nc.sync.dma_start(out=hdt[:, 2304:3072], in_=hd[:, 2304:3072])    # w2
            nc.scalar.dma_start(out=hdt[:, 3072:3840], in_=hd[:, 3072:3840])  # w3
            nc.sync.dma_start(out=hdt[:, 3840:4608], in_=hd[:, 3840:4608])    # w4
            nc.scalar.dma_start(out=hdt[:, 4608:5376], in_=hd[:, 4608:5376])  # w5
            nc.scalar.dma_start(out=xht[0][:], in_=xh[0])
            nc.scalar.dma_start(out=xht[1][:], in_=xh[1])
            # Bias is only needed once vector adds start; PE is
            # unaffected by a late bias (psum depth 4 absorbs it).
            nc.scalar.dma_start(out=bt[:], in_=b[:])
            for i in range(2, N_HEAD - 1):
                nc.scalar.dma_start(out=xht[i][:], in_=xh[i])
            # Final tile's input, needed last; keep it off the block pool.
            xlt = const.tile([P, KC * P], in_dt, tag="xhl", name="xhl")

            def wA(kc):  # W chunk kc, output cols 0:512
                if kc == 0:
                    return hdt[:, 384:896]
                base = 1536 + (kc - 1) * C
                return hdt[:, base : base + 512]

            def wB(kc):  # W chunk kc, output cols 512:768
                if kc == 0:
                    return hdt[:, 1280:1536]
                base = 1536 + (kc - 1) * C + 512
                return hdt[:, base : base + 256]

            # ---- token-tile loop ----
            for g in range(TT):
                if g == 0:

                    def xsl(kc):
                        if kc < 3:
                            return hdt[:, kc * P : (kc + 1) * P]
                        return hdt[:, 896 + (kc - 3) * P : 896 + (kc - 2) * P]
                elif g < N_HEAD:
                    xt = xht[g - 1]

                    def xsl(kc, xt=xt):
                        return xt[:, kc * P : (kc + 1) * P]
                elif g == TT - 1:
                    nc.scalar.dma_start(out=xlt[:], in_=xh[N_HEAD - 1])

                    def xsl(kc):
                        return xlt[:, kc * P : (kc + 1) * P]
                else:
                    bi, s = divmod(g - N_HEAD, TBLK)
                    if s == 0:
                        xbt = xp.tile(
                            [P, KC, TBLK * P], in_dt, tag="xb", name="xb"
                        )
                        nc.scalar.dma_start(
                            out=xbt[:],
                            in_=xb[bi].rearrange(
                                "p (kc t) -> p kc t", kc=KC
                            ),
                        )

                    def xsl(kc, xbt=xbt, s=s):
                        return xbt[:, kc, s * P : (s + 1) * P]

                pt = pp.tile(
                    [P, C], mybir.dt.float32, tag=f"pt{g % 4}", name=f"pt{g % 4}"
                )
                ot = op.tile([P, C], mybir.dt.bfloat16, tag="ot")
                row = slice(g * P, (g + 1) * P)
                if g == TT - 1:
                    # Final tile: 0:512 half computed FIRST so its vector
                    # add + scalar-ring DMA drain while the 512:768 half
                    # is still on the PE; the 512:768 half runs as two
                    # 128-col pieces whose add+DMA chains split across
                    # both rings, minimizing the post-last-matmul path.
                    # The halves use DIFFERENT psum tags so the second
                    # half's matmuls carry no WAR dependency on the adds.
                    pt2 = pp.tile(
                        [P, C], mybir.dt.float32,
                        tag=f"pt{(g + 1) % 4}", name=f"pt{(g + 1) % 4}",
                    )
                    for kc in range(KC):
                        nc.tensor.matmul(
                            pt2[:, 0:512], xsl(kc), wA(kc),
                            start=(kc == 0), stop=(kc == KC - 1),
                        )
                    nc.vector.tensor_add(
                        out=ot[:, 0:512], in0=pt2[:, 0:512], in1=bt[:, 0:512]
                    )
                    nc.scalar.dma_start(out=out[row, 0:512], in_=ot[:, 0:512])
                    # 512:768 must be ONE accumulation group (both 128-col
                    # sub-ranges share a PSUM bank == one zero region; two
                    # groups there are illegal).  Split only the post-stop
                    # add+DMA chains across both rings.
                    for kc in range(KC):
                        nc.tensor.matmul(
                            pt[:, 512:C], xsl(kc), wB(kc),
                            start=(kc == 0), stop=(kc == KC - 1),
                        )
                    nc.vector.tensor_add(
                        out=ot[:, 512:640], in0=pt[:, 512:640], in1=bt[:, 512:640]
                    )
                    nc.sync.dma_start(out=out[row, 512:640], in_=ot[:, 512:640])
                    nc.vector.tensor_add(
                        out=ot[:, 640:768], in0=pt[:, 640:768], in1=bt[:, 640:768]
                    )
                    nc.scalar.dma_start(out=out[row, 640:768], in_=ot[:, 640:768])
                    continue
                for kc in range(KC):
                    lhsT = xsl(kc)
                    nc.tensor.matmul(
                        pt[:, 0:512], lhsT, wA(kc),
                        start=(kc == 0), stop=(kc == KC - 1),
                    )
                    nc.tensor.matmul(
                        pt[:, 512:C], lhsT, wB(kc),
                        start=(kc == 0), stop=(kc == KC - 1),
                    )

                if g == TT - 2:
                    # Tail drain: per-half add + DMA, halves split across
                    # BOTH rings so issue (~0.6us/instr) and completion
                    # receipts run in parallel.
                    nc.vector.tensor_add(
                        out=ot[:, 0:512], in0=pt[:, 0:512], in1=bt[:, 0:512]
                    )
                    nc.scalar.dma_start(out=out[row, 0:512], in_=ot[:, 0:512])
                    nc.vector.tensor_add(
                        out=ot[:, 512:C], in0=pt[:, 512:C], in1=bt[:, 512:C]
                    )
                    nc.sync.dma_start(out=out[row, 512:C], in_=ot[:, 512:C])
                else:
                    # split at the PSUM bank boundary (one bank per read)
                    nc.vector.tensor_add(
                        out=ot[:, 0:512], in0=pt[:, 0:512], in1=bt[:, 0:512]
                    )
                    nc.vector.tensor_add(
                        out=ot[:, 512:C], in0=pt[:, 512:C], in1=bt[:, 512:C]
                    )
                    nc.sync.dma_start(out=out[row, :], in_=ot[:])
    nc.compile()
    return nc


def _fold_weights(qkv_w, qkv_b, proj_w, proj_b, pe):
    v_w = qkv_w[2 * 4 : 3 * 4].astype(np.float64)   # [4, 4]
    v_b = qkv_b[2 * 4 : 3 * 4].astype(np.float64)   # [4]
    bd = np.kron(np.eye(C // 4), v_w.T)             # y_flat @ bd == groupwise v
    w_eff = bd @ proj_w.astype(np.float64).T        # [768, 768]
    b_eff = (
        np.tile(v_b, C // 4) @ proj_w.astype(np.float64).T
        + proj_b.astype(np.float64)
        + pe[:C].astype(np.float64) @ w_eff
    )
    return w_eff, b_eff


def _enable_tracing_shims():
    """Dev-only (GWTA_TRACE=1): restore the NTFF profile hook that this
    image's `antenv` is missing, and keep trace artifacts local instead of
    uploading.  Never active when the kernel is called normally."""
    import sys
    import types

    try:
        from antenv import axon_hooks  # noqa: F401
    except ImportError:
        import antenv
        from trn_agent_boot.trn_boot import _ntff_profile_via_ctypes

        mod = types.ModuleType("antenv.axon_hooks")
        mod._hook = _ntff_profile_via_ctypes("/opt/axon/libaxon_pjrt.so")
        mod.get_axon_ntff_profile_hook = lambda: mod._hook
        mod.set_axon_ntff_profile_hook = lambda h: setattr(mod, "_hook", h)
        sys.modules["antenv.axon_hooks"] = mod
        antenv.axon_hooks = mod

    import concourse.bass_utils as bu

    bu.upload_artifacts = lambda tmpdir: f"local:{tmpdir}"


def kernel(x, qkv_w, qkv_b, proj_w, proj_b, pe):
    x = np.asarray(x, np.float32)
    w_eff, b_eff = _fold_weights(
        np.asarray(qkv_w), np.asarray(qkv_b),
        np.asarray(proj_w), np.asarray(proj_b), np.asarray(pe),
    )

    variant = VARIANT
    if variant == "bf16":
        cast = lambda a: np.ascontiguousarray(a, dtype=ml_dtypes.bfloat16)
    else:
        cast = lambda a: np.ascontiguousarray(a, dtype=np.float32)

    # W packed partition-major: (p, kc, j) = W_eff[kc*128+p, j]
    w_dev = np.ascontiguousarray(
        cast(w_eff).reshape(KC, P, C).transpose(1, 0, 2)
    ).reshape(P, KC * C)
    b_dev = np.broadcast_to(
        b_eff.astype(ml_dtypes.bfloat16), (P, C)
    ).copy()

    x_flat = x.reshape(ROWS, C)
    in_maps = []
    head_tiles = list(range(1, N_HEAD)) + [TT - 1]
    for c in range(N_CORES):
        xT = cast(x_flat[c * RPC : (c + 1) * RPC].T)   # [C, RPC]
        xr = xT.reshape(KC, P, RPC)
        t0p = np.ascontiguousarray(
            xr[:, :, 0:P].transpose(1, 0, 2)
        ).reshape(P, KC * P)
        hd_dev = np.ascontiguousarray(
            np.concatenate(
                [
                    t0p[:, 0:384],
                    w_dev[:, 0:512],
                    t0p[:, 384:768],
                    w_dev[:, 512:768],
                    w_dev[:, 768:],
                ],
                axis=1,
            )
        )
        xh_dev = np.ascontiguousarray(
            np.stack(
                [xr[:, :, t * P : (t + 1) * P] for t in head_tiles], axis=0
            ).transpose(0, 2, 1, 3)
        ).reshape(N_HEAD, P, KC * P)
        xb_dev = np.ascontiguousarray(
            xr[:, :, N_HEAD * P : (TT - 1) * P]
            .reshape(KC, P, NBLK, TBLK * P)
            .transpose(2, 1, 0, 3)
        ).reshape(NBLK, P, KC * TBLK * P)
        in_maps.append(
            {"hd": hd_dev, "xh": xh_dev, "xb": xb_dev, "b": b_dev}
        )

    nc = _build_nc(variant)
    trace = bool(int(os.environ.get("GWTA_TRACE", "0")))
    kw = {}
    if trace:
        _enable_tracing_shims()
        kw["tmpdir"] = os.environ.get("GWTA_TRACE_DIR") or None
    r = run_bass_kernel_spmd(nc, in_maps, list(range(N_CORES)), trace=trace, **kw)

    LAST_STATS.clear()
    LAST_STATS.update(
        exec_time_ns=r.exec_time_ns,
        mean_exec_time_ns=r.mean_exec_time_ns,
        variant=variant,
    )

    out = np.empty((ROWS, C), np.float32)
    for c in range(N_CORES):
        out[c * RPC : (c + 1) * RPC] = np.asarray(
            r.results[c]["out"]
        ).astype(np.float32)
    return out.reshape(B, H, W, C)
